# revision 8
# baseline (speedup 1.0000x reference)
"""GraphSAGE layer kernel for Trainium2, SPMD over 8 NeuronCores.

Math (per reference):
    x3   = inputs.reshape(B, N, D)                      # B=128, N=4096, D=32
    out  = relu(x3 @ W_self + (A^T @ (x3 @ W_neigh)))   # per batch
    out  = out.reshape(B, N*D)

Strategy (v2, fp8 DoubleRow):
  - Pure data-parallel over batch: 16 batches per core.
  - The dominant A @ T aggregation runs in fp8 (e4m3) with the PE's
    DoubleRow perf mode: each matmul contracts K=256 (two 128-row A
    blocks packed per partition) at 0.5 cycles per output column — 4x
    the fp16 MAC rate. A is host-prescaled by S=4096 so its ~1/N-sized
    entries land in e4m3's normal range; the final ReLU evacuation
    rescales by 1/S on the ScalarE (out = relu(psum * 1/S)).
  - The neighbor part is ~2% of the output's magnitude (A is a mean
    aggregator: |A@T| ~ sqrt(4/(3N)) vs |X@Ws| ~ 1), so e4m3
    quantization of A and T contributes only ~1e-3 relative RMS error.
  - The self part X@W_self needs full accuracy and is computed in fp16
    directly into the same PSUM accumulation: 4 extra matmuls per
    output block with a block-diagonal W_self*S stationary, using the
    SBUF-resident XT. No separate T2 tensor, no DVE add.
  - Transform phase computes only T = X@W_neigh (block-diag weights,
    4 batches per 128-wide K), evacuated straight to fp8 with DVE/ACT
    alternating per i-block.
  - Host-side layouts: XT [128=(b%4)*32+p, (ib, b//4, i%128)] fp16;
    A pretransposed to DoubleRow pairs [i%128, (jb, ibp, two, j%128)]
    fp8 scaled by S; output written as [j, (b_loc, q)] fp16 and
    untransposed/upcast on the host.
"""

import numpy as np

B, N, D = 128, 4096, 32
NCORES = 8
BSH = B // NCORES          # 16 batches per core
NIB = N // 128             # 32 node blocks
NPAIR = NIB // 2           # 16 DoubleRow pairs of i-blocks
NB4 = BSH // 4             # 4 groups of 4 batches
BQ = BSH * D               # 512 = free width of the aggregation psum
S = 4096.0                 # fp8 scale carried by A and W_self

_CACHE = {}


def _build_program():
    import concourse.bacc as bacc
    import concourse.mybir as mybir
    import concourse.tile as tile
    from contextlib import ExitStack

    f32 = mybir.dt.float32
    fp16 = mybir.dt.float16
    fp8 = mybir.dt.float8e4
    DR = mybir.MatmulPerfMode.DoubleRow
    Relu = mybir.ActivationFunctionType.Relu

    nc = bacc.Bacc(
        trn_type="TRN2", target_bir_lowering=False, debug=False, num_devices=NCORES
    )
    xt = nc.dram_tensor("xt", [128, NIB * NB4 * 128], fp16, kind="ExternalInput").ap()
    # bd2: cols 0:128 blockdiag(4 x W_neigh); cols 128:256 blockdiag(4 x W_self*S)
    bd2 = nc.dram_tensor("bd2", [128, 256], fp16, kind="ExternalInput").ap()
    a8 = nc.dram_tensor(
        "a8", [128, NIB * NPAIR * 2 * 128], fp8, kind="ExternalInput"
    ).ap()
    y = nc.dram_tensor("y", [N, BQ], fp16, kind="ExternalOutput").ap()

    with tile.TileContext(nc) as tc, ExitStack() as ctx:
        const_pool = ctx.enter_context(tc.tile_pool(name="const", bufs=1))
        xt_pool = ctx.enter_context(tc.tile_pool(name="xtp", bufs=1))
        t_pool = ctx.enter_context(tc.tile_pool(name="tp", bufs=1))
        a_pool = ctx.enter_context(tc.tile_pool(name="ap", bufs=8))
        out_pool = ctx.enter_context(tc.tile_pool(name="op", bufs=4))
        pt_pool = ctx.enter_context(tc.tile_pool(name="ptp", bufs=5, space="PSUM"))
        po_pool = ctx.enter_context(tc.tile_pool(name="pop", bufs=3, space="PSUM"))

        bd2_sb = const_pool.tile([128, 256], fp16)
        # scalar queue: its DGE init overlaps the sync queue's first XT chunk
        nc.scalar.dma_start(bd2_sb[:], bd2[:])

        # XT resident in SBUF: [128, ib, b4, il] (32 KB/partition)
        xt_sb = xt_pool.tile([128, NIB, NB4, 128], fp16)
        xt_r = xt.rearrange("p (ib b4 il) -> p ib b4 il", ib=NIB, b4=NB4)
        XTCH = 2  # i-blocks per XT chunk DMA (256 KB each)
        for c in range(NIB // XTCH):
            nc.sync.dma_start(
                xt_sb[:, c * XTCH : (c + 1) * XTCH, :, :],
                xt_r[:, c * XTCH : (c + 1) * XTCH, :, :],
            )

        # T in fp8: [i%128, (ib, b, q)] = [128, NIB*512] (16 KB/partition)
        t8 = t_pool.tile([128, NIB * BQ], fp8)
        t8_r = t8.rearrange("p (ib n) -> p ib n", ib=NIB)
        t8_dr = t8.rearrange("p (ibp two n) -> p ibp two n", ibp=NPAIR, two=2)

        # a8 host layout: [p, (jb, ibp, two, j)]
        a8_r = a8.rearrange(
            "p (jb ibp two j) -> p jb ibp two j", jb=NIB, ibp=NPAIR, two=2
        )

        # ---- transform: T = X @ W_neigh via block-diag weights ----
        for ib in range(NIB):
            pt = pt_pool.tile([128, NB4, 128], f32, tag="pt", name=f"pt{ib}")
            for b4 in range(NB4):
                nc.tensor.matmul(
                    pt[:, b4, :],
                    xt_sb[:, ib, b4, :],
                    bd2_sb[:, 0:128],
                    start=(b4 == 0),
                    stop=(b4 == NB4 - 1),
                )
            # pt[il, (b4, bh, qn)] -> t8[il, ib, (b, q)]: flat contiguous copy
            src = pt.rearrange("p b4 j -> p (b4 j)")
            if ib % 2 == 0:
                nc.vector.tensor_copy(t8_r[:, ib, :], src)
            else:
                nc.scalar.copy(t8_r[:, ib, :], src)

        # ---- aggregation + self-part + relu, one j-block at a time ----
        for jb in range(NIB):
            a_t = a_pool.tile([128, NPAIR, 2, 128], fp8, tag="a", name=f"a{jb}")
            nc.sync.dma_start(a_t[:], a8_r[:, jb])
            po = po_pool.tile([128, BQ], f32, tag="po", name=f"po{jb}")
            for ibp in range(NPAIR):
                for h in range(2):
                    nc.tensor.matmul(
                        po[:, h * 256 : (h + 1) * 256],
                        a_t[:, ibp, :, :],
                        t8_dr[:, ibp, :, h * 256 : (h + 1) * 256],
                        start=(ibp == 0 and h == 0),
                        stop=False,
                        perf_mode=DR,
                    )
            # self part: po[:, b4*128:+128] += XT[:, jb, b4, :].T @ blockdiag(Ws*S)
            for b4 in range(NB4):
                nc.tensor.matmul(
                    po[:, b4 * 128 : (b4 + 1) * 128],
                    xt_sb[:, jb, b4, :],
                    bd2_sb[:, 128:256],
                    start=False,
                    stop=(b4 == NB4 - 1),
                )
            ob = out_pool.tile([128, BQ], fp16, tag="ob", name=f"ob{jb}")
            nc.scalar.activation(ob[:], po[:], Relu, scale=1.0 / S)
            # last block: SP queue is idle by then and has lower issue latency
            dma_eng = nc.sync if jb == NIB - 1 else nc.scalar
            dma_eng.dma_start(y[jb * 128 : (jb + 1) * 128, :], ob[:])

    nc.compile()
    return nc


def _get_program():
    if "nc" not in _CACHE:
        _CACHE["nc"] = _build_program()
    return _CACHE["nc"]


def make_in_maps(x3, adj, W_neigh, W_self):
    import ml_dtypes

    # bd2: [blockdiag(4 x Wn) | blockdiag(4 x Ws*S)]
    bd2 = np.zeros((128, 256), dtype=np.float32)
    for bh in range(4):
        bd2[bh * 32 : (bh + 1) * 32, bh * 32 : (bh + 1) * 32] = W_neigh
        bd2[bh * 32 : (bh + 1) * 32, 128 + bh * 32 : 128 + (bh + 1) * 32] = W_self * S
    bd2 = bd2.astype(np.float16)

    # A -> [p, (jb, ibp, two, j)] scaled by S, fp8 e4m3
    a8 = np.ascontiguousarray(
        (adj * S).reshape(NPAIR, 2, 128, NIB, 128).transpose(2, 3, 0, 1, 4)
    ).reshape(128, NIB * NPAIR * 2 * 128).astype(ml_dtypes.float8_e4m3)

    in_maps = []
    for c in range(NCORES):
        xs = x3[c * BSH : (c + 1) * BSH]          # [16, N, 32]
        # XT[(bh*32+p), (ib, b4, il)] = xs[b4*4 + bh, ib*128 + il, p]
        xt = np.ascontiguousarray(
            xs.reshape(NB4, 4, NIB, 128, D).transpose(1, 4, 2, 0, 3)
        ).reshape(128, NIB * NB4 * 128).astype(np.float16)
        in_maps.append({"xt": xt, "bd2": bd2, "a8": a8})
    return in_maps


def kernel(inputs, adj, W_neigh, W_self, batch_train=None):
    from concourse.bass_utils import run_bass_kernel_spmd

    inputs = np.asarray(inputs, dtype=np.float32)
    adj = np.ascontiguousarray(np.asarray(adj, dtype=np.float32))
    W_neigh = np.asarray(W_neigh, dtype=np.float32)
    W_self = np.asarray(W_self, dtype=np.float32)

    x3 = inputs.reshape(B, N, D)
    in_maps = make_in_maps(x3, adj, W_neigh, W_self)

    nc = _get_program()
    res = run_bass_kernel_spmd(nc, in_maps, list(range(NCORES)))

    out = np.empty((B, N * D), dtype=np.float32)
    for c in range(NCORES):
        yc = np.asarray(res.results[c]["y"], dtype=np.float32)  # [j, (b_loc, q)]
        out[c * BSH : (c + 1) * BSH] = (
            yc.reshape(N, BSH, D).transpose(1, 0, 2).reshape(BSH, N * D)
        )
    return out
